# revision 1
# baseline (speedup 1.0000x reference)
"""Bass/Trainium2 kernel for nn_BBBGraphConv (Bayesian GraphConv, DGL norm='both').

Computation (reference):
    W    = W_mu + W_eps * softplus(W_rho)
    bias = bias_mu + bias_eps * softplus(bias_rho)
    o    = clip(out_deg, 1)^-0.5 ; i = clip(in_deg, 1)^-0.5
    out  = segsum_dst((feat * o)[src]) @ W * i[:, None] + bias

Distribution: edges are bucketed by destination node; each of the 8 cores owns a
contiguous range of 12544 destination nodes (98 blocks x 128 dsts) and computes
its output rows exclusively (no all-reduce needed). The (pre-scaled) node
feature table is replicated to every core. Each core gathers the source rows of
its own edges with the gpsimd dma_gather instruction (the src id space is split
into 4 windows of 25088 rows so indices fit dma_gather's int16 format), reduces
them per destination with a TensorE one-hot-mask matmul (mask built from an
iota/is_equal compare on VectorE), projects through W, applies the dst-side
norm + bias, and writes its slice of the output.

Host-side work is limited to index-domain preprocessing (degree counts, sort,
bucketing/padding) and the out-degree pre-scaling of the feature table.
"""

import numpy as np
from contextlib import ExitStack

import concourse.bass as bass
import concourse.bacc as bacc
import concourse.tile as tile
from concourse import mybir
from concourse.bass_utils import run_bass_kernel_spmd

# Problem constants (hardcoded per the harness contract)
N_NODES = 100_000
N_EDGES = 1_600_000
C = 128          # in_ch == out_ch
P = 128          # partitions
N_CORES = 8
BLK = 128        # dst nodes per block
NB = 98          # blocks per core
D_CORE = NB * BLK          # 12544 dst rows per core
N_PAD = N_CORES * D_CORE   # 100352

NW = 4           # src windows (dma_gather indices are int16)
WROWS = N_PAD // NW        # 25088 rows per window


def _sb_layout(s_cap: int):
    """Blocks per (superblock, window) gather call. k*s_cap is capped at 35
    (num_idxs <= 4480 per dma_gather call, the hardware-validated size) and the
    tail is tapered so the post-gather compute tail is short."""
    k = max(1, min(7, 35 // s_cap))
    sizes = [k] * (NB // k)
    rem = NB - k * len(sizes)
    if rem:
        sizes.append(rem)
    tail = sizes.pop()
    while tail > 1:
        h = tail // 2
        sizes.append(tail - h)
        tail = h
    sizes.append(1)
    assert sum(sizes) == NB
    offs = [sum(sizes[:i]) for i in range(len(sizes))]
    return sizes, offs

TBL_DT = mybir.dt.float16
TBL_NP = np.float16

_CACHE: dict = {}


def _build_program(s_cap: int):
    """Build the SPMD Bass program (one graph, runs on all 8 cores).

    s_cap: groups (of 128 edge slots) per (block, window) section.
    """
    gpb = NW * s_cap            # groups per block
    slots_blk = gpb * BLK       # edge slots per block
    idx_f_blk = s_cap * BLK // 16        # idx free-dim per (block, window)
    idx_f_total = NB * NW * idx_f_blk
    SB_SIZES, SB_OFF = _sb_layout(s_cap)
    N_SB = len(SB_SIZES)
    f32 = mybir.dt.float32

    nc = bacc.Bacc("TRN2", target_bir_lowering=False, debug=False, num_swdge_queues=4)

    table = nc.dram_tensor("table", [N_PAD, C], TBL_DT, kind="ExternalInput").ap()
    idx_t = nc.dram_tensor("idx", [P, idx_f_total], mybir.dt.int16,
                           kind="ExternalInput").ap()
    rel_t = nc.dram_tensor("rel", [P, NB * gpb], TBL_DT, kind="ExternalInput").ap()
    iota_t = nc.dram_tensor("iota", [P, gpb * BLK], TBL_DT, kind="ExternalInput").ap()
    ivec_t = nc.dram_tensor("ivec", [P, NB], f32, kind="ExternalInput").ap()
    w_mu = nc.dram_tensor("w_mu", [C, C], f32, kind="ExternalInput").ap()
    w_rho = nc.dram_tensor("w_rho", [C, C], f32, kind="ExternalInput").ap()
    w_eps = nc.dram_tensor("w_eps", [C, C], f32, kind="ExternalInput").ap()
    b_mu = nc.dram_tensor("b_mu", [1, C], f32, kind="ExternalInput").ap()
    b_rho = nc.dram_tensor("b_rho", [1, C], f32, kind="ExternalInput").ap()
    b_eps = nc.dram_tensor("b_eps", [1, C], f32, kind="ExternalInput").ap()
    out = nc.dram_tensor("out", [D_CORE, C], f32, kind="ExternalOutput").ap()

    with tile.TileContext(nc) as tc, ExitStack() as ctx:
        const = ctx.enter_context(tc.tile_pool(name="const", bufs=1))
        gpool = ctx.enter_context(tc.tile_pool(name="gather", bufs=3))
        mpool = ctx.enter_context(tc.tile_pool(name="mask", bufs=3))
        apool = ctx.enter_context(tc.tile_pool(name="aggf", bufs=3))
        opool = ctx.enter_context(tc.tile_pool(name="ostage", bufs=3))
        pa_pool = ctx.enter_context(tc.tile_pool(name="pa", bufs=3, space="PSUM"))
        pb_pool = ctx.enter_context(tc.tile_pool(name="pb", bufs=2, space="PSUM"))
        pc_pool = ctx.enter_context(tc.tile_pool(name="pc", bufs=1, space="PSUM"))

        # --- resident inputs -------------------------------------------------
        # idx loaded per superblock (separate tiles) so the first gather
        # doesn't wait for the whole index upload
        idx_off = [0]
        for k in SB_SIZES:
            idx_off.append(idx_off[-1] + NW * k * idx_f_blk)
        idx_tiles = []
        for s in range(N_SB):
            t = const.tile([P, idx_off[s + 1] - idx_off[s]], mybir.dt.int16,
                           tag=f"idx{s}")
            nc.sync.dma_start(out=t[:], in_=idx_t[:, idx_off[s]:idx_off[s + 1]])
            idx_tiles.append(t)
        rel_sb = const.tile([P, NB * gpb], TBL_DT, tag="rel")
        nc.sync.dma_start(out=rel_sb[:], in_=rel_t[:])
        ivec_sb = const.tile([P, NB], f32, tag="ivec")
        nc.sync.dma_start(out=ivec_sb[:], in_=ivec_t[:])

        # --- W = W_mu + W_eps * softplus(W_rho) ------------------------------
        wmu_sb = const.tile([C, C], f32, tag="wmu")
        nc.sync.dma_start(out=wmu_sb[:], in_=w_mu[:])
        wrho_sb = const.tile([C, C], f32, tag="wrho")
        nc.sync.dma_start(out=wrho_sb[:], in_=w_rho[:])
        weps_sb = const.tile([C, C], f32, tag="weps")
        nc.sync.dma_start(out=weps_sb[:], in_=w_eps[:])
        w_sp = const.tile([C, C], f32, tag="wsp")
        nc.scalar.activation(w_sp[:], wrho_sb[:], mybir.ActivationFunctionType.Exp)
        nc.scalar.activation(w_sp[:], w_sp[:], mybir.ActivationFunctionType.Ln, bias=1.0)
        w_sb = const.tile([C, C], f32, tag="w")
        nc.vector.tensor_tensor(out=w_sb[:], in0=weps_sb[:], in1=w_sp[:], op=mybir.AluOpType.mult)
        nc.vector.tensor_tensor(out=w_sb[:], in0=w_sb[:], in1=wmu_sb[:], op=mybir.AluOpType.add)

        # --- bias tile [P, C]: every partition row holds the bias vector -----
        bmu_sb = const.tile([1, C], f32, tag="bmu")
        nc.sync.dma_start(out=bmu_sb[:], in_=b_mu[:])
        brho_sb = const.tile([1, C], f32, tag="brho")
        nc.sync.dma_start(out=brho_sb[:], in_=b_rho[:])
        beps_sb = const.tile([1, C], f32, tag="beps")
        nc.sync.dma_start(out=beps_sb[:], in_=b_eps[:])
        b_sp = const.tile([1, C], f32, tag="bsp")
        nc.scalar.activation(b_sp[:], brho_sb[:], mybir.ActivationFunctionType.Exp)
        nc.scalar.activation(b_sp[:], b_sp[:], mybir.ActivationFunctionType.Ln, bias=1.0)
        b_vec = const.tile([1, C], f32, tag="bvec")
        nc.vector.tensor_tensor(out=b_vec[:], in0=beps_sb[:], in1=b_sp[:], op=mybir.AluOpType.mult)
        nc.vector.tensor_tensor(out=b_vec[:], in0=b_vec[:], in1=bmu_sb[:], op=mybir.AluOpType.add)
        ones_1p = const.tile([1, C], f32, tag="ones")
        nc.vector.memset(ones_1p[:], 1.0)
        p_bias = pc_pool.tile([P, C], f32, tag="pbias")
        nc.tensor.matmul(out=p_bias[:], lhsT=ones_1p[:], rhs=b_vec[:], start=True, stop=True)
        bias_tile = const.tile([P, C], f32, tag="bias")
        nc.vector.tensor_copy(out=bias_tile[:], in_=p_bias[:])

        # --- iota over d within a block (host-provided constant) -------------
        iota_m = const.tile([P, slots_blk], TBL_DT, tag="iotam")
        nc.sync.dma_start(out=iota_m[:], in_=iota_t[:])
        iota3 = iota_m[:].rearrange("p (g d) -> p g d", g=gpb)

        # --- main loop over superblocks --------------------------------------
        for s in range(N_SB):
            k_sb = SB_SIZES[s]
            sb_groups = k_sb * s_cap
            g_tile = gpool.tile([P, sb_groups * NW * C], TBL_DT, tag="g")
            g3 = g_tile[:].rearrange("p (g c) -> p g c", c=C)
            for w in range(NW):
                call = w * k_sb * idx_f_blk
                nc.gpsimd.dma_gather(
                    out_ap=g3[:, w * sb_groups:(w + 1) * sb_groups, :],
                    in_ap=table[w * WROWS:(w + 1) * WROWS, :],
                    idxs_ap=idx_tiles[s][:, call:call + k_sb * idx_f_blk],
                    num_idxs=sb_groups * BLK,
                    num_idxs_reg=sb_groups * BLK,
                    elem_size=C,
                    queue_num=w,
                    single_packet=False,
                )
            ostage = opool.tile([P, k_sb * C], f32, tag="ostage")
            for bb in range(k_sb):
                b = SB_OFF[s] + bb
                mask = mpool.tile([P, slots_blk], TBL_DT, tag="mask")
                rel_b = rel_sb[:, b * gpb:(b + 1) * gpb].unsqueeze(2).to_broadcast(
                    [P, gpb, BLK]
                )
                nc.vector.tensor_tensor(
                    out=mask[:].rearrange("p (g d) -> p g d", g=gpb),
                    in0=iota3,
                    in1=rel_b,
                    op=mybir.AluOpType.is_equal,
                )
                pa = pa_pool.tile([C, BLK], f32, tag="pa")
                for j in range(gpb):
                    w, g = divmod(j, s_cap)
                    gsl = (w * k_sb + bb) * s_cap + g
                    nc.tensor.matmul(
                        out=pa[:],
                        lhsT=g_tile[:, gsl * C:(gsl + 1) * C],
                        rhs=mask[:, j * BLK:(j + 1) * BLK],
                        start=(j == 0),
                        stop=(j == gpb - 1),
                    )
                agg = apool.tile([C, BLK], f32, tag="agg")
                nc.scalar.activation(agg[:], pa[:], mybir.ActivationFunctionType.Copy)
                pb = pb_pool.tile([BLK, C], f32, tag="pb")
                nc.tensor.matmul(out=pb[:], lhsT=agg[:], rhs=w_sb[:], start=True, stop=True)
                nc.vector.scalar_tensor_tensor(
                    out=ostage[:, bb * C:(bb + 1) * C],
                    in0=pb[:],
                    scalar=ivec_sb[:, b:b + 1],
                    in1=bias_tile[:],
                    op0=mybir.AluOpType.mult,
                    op1=mybir.AluOpType.add,
                )
            dram_view = out[SB_OFF[s] * BLK:(SB_OFF[s] + k_sb) * BLK, :].rearrange(
                "(bb p) c -> p bb c", p=P
            )
            nc.sync.dma_start(
                out=dram_view, in_=ostage[:].rearrange("p (bb c) -> p bb c", bb=k_sb)
            )

    nc.compile()
    return nc


def _preprocess(feat, src, dst, W_mu, W_rho, bias_mu, bias_rho, W_eps, bias_eps):
    """Index-domain preprocessing + table pre-scaling. Returns per-core in_maps."""
    src = np.asarray(src).astype(np.int64)
    dst = np.asarray(dst).astype(np.int64)
    feat = np.asarray(feat, dtype=np.float32)

    out_deg = np.bincount(src, minlength=N_NODES).astype(np.float32)
    o = 1.0 / np.sqrt(np.maximum(out_deg, 1.0))
    in_deg = np.bincount(dst, minlength=N_NODES)
    ivec_full = (1.0 / np.sqrt(np.maximum(in_deg, 1.0))).astype(np.float32)

    table = np.zeros((N_PAD, C), TBL_NP)
    table[:N_NODES] = (feat * o[:, None]).astype(TBL_NP)

    blk = dst >> 7                      # global dst block, 0..783
    win = src // WROWS                  # src window, 0..3
    order = np.lexsort((src, win + np.int64(NW) * blk))
    sblk = blk[order]
    swin = win[order]
    ss = src[order]
    sd = dst[order]

    n_blocks = N_CORES * NB
    sec = sblk * NW + swin              # global (block, window) section id
    sec_cnt = np.bincount(sec, minlength=n_blocks * NW)
    s_cap = int(np.ceil(sec_cnt.max() / BLK))
    sec_slots = s_cap * BLK

    starts = np.zeros(n_blocks * NW + 1, np.int64)
    np.cumsum(sec_cnt, out=starts[1:])
    pos = np.arange(len(ss), dtype=np.int64) - starts[sec]

    # slot id in (core, superblock, window, block-in-sb, group, lane) order
    SB_SIZES, SB_OFF = _sb_layout(s_cap)
    s_of_b = np.zeros(NB, np.int64)
    b_in_of_b = np.zeros(NB, np.int64)
    for s, k in enumerate(SB_SIZES):
        s_of_b[SB_OFF[s]:SB_OFF[s] + k] = s
        b_in_of_b[SB_OFF[s]:SB_OFF[s] + k] = np.arange(k)
    sb_off_arr = np.asarray(SB_OFF, np.int64)
    k_arr = np.asarray(SB_SIZES, np.int64)
    core = sblk // NB
    b_loc = sblk % NB
    s_id = s_of_b[b_loc]
    sec_idx = NW * sb_off_arr[s_id] + swin * k_arr[s_id] + b_in_of_b[b_loc]
    slots_core = NB * NW * sec_slots
    slot = core * slots_core + sec_idx * sec_slots + pos

    idx_all = np.zeros(N_CORES * slots_core, np.int16)    # pad -> row 0 of window
    idx_all[slot] = (ss - swin * WROWS).astype(np.int16)

    # rel in (core, block, window, group, lane) order
    rel_slot = (sblk * NW + swin) * sec_slots + pos
    rel_all = np.full(n_blocks * NW * sec_slots, -1.0, TBL_NP)
    rel_all[rel_slot] = (sd & 127).astype(TBL_NP)

    ivec_pad = np.ones(N_PAD, np.float32)
    ivec_pad[:N_NODES] = ivec_full

    gpb = NW * s_cap
    iota_np = np.tile(np.arange(BLK, dtype=TBL_NP), gpb)[None, :].repeat(P, 0)
    iota_np = np.ascontiguousarray(iota_np)

    cc = np.ascontiguousarray
    in_maps = []
    for c in range(N_CORES):
        # idx: 16-wrapped int16 (valid per call since call sizes are multiples
        # of 16), replicated across the 8 Q7 pairs
        idx_c = idx_all[c * slots_core:(c + 1) * slots_core]
        idx_tile = np.tile(idx_c.reshape(-1, 16).T, (8, 1))
        rel_c = rel_all[c * NB * NW * sec_slots:(c + 1) * NB * NW * sec_slots]
        in_maps.append({
            "table": table,
            "iota": iota_np,
            "idx": cc(idx_tile),
            "rel": cc(rel_c.reshape(-1, P).T),
            "ivec": cc(ivec_pad[c * D_CORE:(c + 1) * D_CORE].reshape(NB, P).T),
            "w_mu": np.asarray(W_mu, np.float32),
            "w_rho": np.asarray(W_rho, np.float32),
            "w_eps": np.asarray(W_eps, np.float32),
            "b_mu": np.asarray(bias_mu, np.float32).reshape(1, C),
            "b_rho": np.asarray(bias_rho, np.float32).reshape(1, C),
            "b_eps": np.asarray(bias_eps, np.float32).reshape(1, C),
        })
    return in_maps, s_cap


def kernel(**inputs) -> np.ndarray:
    in_maps, s_cap = _preprocess(**inputs)
    if s_cap not in _CACHE:
        _CACHE[s_cap] = _build_program(s_cap)
    nc = _CACHE[s_cap]
    res = run_bass_kernel_spmd(nc, in_maps, core_ids=list(range(N_CORES)))
    parts = [res.results[c]["out"] for c in range(N_CORES)]
    return np.concatenate(parts, axis=0)[:N_NODES]

